# revision 1
# baseline (speedup 1.0000x reference)
"""Trainium2 Bass kernel for nn_ClosedFormLoss (closed-form matting Laplacian loss).

Math: the reference computes, per (batch, class), y = keep * (vals @ ow) per 3x3
window followed by a scatter-add, where vals is the 9x9 matting-Laplacian block
built from a per-window 3x3 color covariance inverse.  Expanded per window w and
pixel p = w+k:
    (vals @ ow)_k = ow_k - (1/9) * (S(w) + (imgn(p) - mu(w)) . v(w))
with S = box3(o), q_ch = box3(imgn_ch * o), t = q - mu*S, v = inv * t.
The scatter-add (windows -> pixels) is the transpose box filter, so
    Lo(p) = o(p)*Nk(p) - A(p) - sum_ch imgn_ch(p) * B_ch(p)
    A  = box3T(keep/9 * (S - mu.v)),  B_ch = box3T(keep/9 * v_ch),
    Nk = box3T(keep)
and loss_bc = sum_p (Lo + conf*o - tri*conf)^2 / n^2,  n = H*W.

All box filters run on the tensor engine as banded-matrix matmuls (partition dim)
accumulated over 3 column shifts (free dim).  Everything else is elementwise on
DVE/ACT.  Sharding: 8 cores = 2 batches x 4 row-quarters; each core processes its
quarter as two 64/65-row sub-stripes and emits one partial sum of squares.
"""

import sys
import numpy as np

sys.path.insert(0, "/opt/trn_rl_repo")

from concourse import bacc, mybir, tile  # noqa: E402
from concourse.bass_utils import run_bass_kernel_spmd  # noqa: E402

F32 = mybir.dt.float32
BF16 = mybir.dt.bfloat16
I32 = mybir.dt.int32
OP = mybir.AluOpType

N_CLASSES = 7
H = W = 513
NWC = 511          # window grid cols
N = H * W
EPS9 = 1e-7 / 9.0
TRI_CONF = 100.0
# keep9 is stored in bf16; 9_eff undoes the bf16 rounding of 1/9 exactly
import ml_dtypes  # noqa: E402
NINE_EFF = float(1.0 / np.float32(np.float32(1.0 / 9.0).astype(ml_dtypes.bfloat16)))

# 8 sub-stripes per batch image; sub s owns output pixel rows [64s, 64s+64)
# (sub 7 owns [448, 513)).  Each core handles one batch and two subs.
SUB_R0 = [64 * s for s in range(8)]
N_OUT = 65          # rows computed per sub (65 with 1-row overlap)
N_WIN = 67          # window rows per sub: [r0-2, r0+65)
N_PIX = 69          # pixel rows for box inputs: [r0-2, r0+67)
N_M = 71            # trimap rows for dilate: [r0-3, r0+68)
# local row l of every pixel/window-aligned tile <-> global row r0-2+l
# output pixel rows live at local rows [2, 67)
CH0, CH1 = 257, 256  # column chunks for 513-wide transpose-box outputs

_PROGRAM = None


def _build_program():
    nc = bacc.Bacc("TRN2", target_bir_lowering=False, debug=False, num_devices=8)

    cpr_d = nc.declare_dram_parameter("cpr", [2, N_CLASSES, N_PIX, W], F32, isOutput=False)
    img_d = nc.declare_dram_parameter("img", [2, 3, N_PIX, W], F32, isOutput=False)
    trim_d = nc.declare_dram_parameter("trim", [2, N_M, W], I32, isOutput=False)
    band_d = nc.declare_dram_parameter("band", [N_M, N_PIX], F32, isOutput=False)
    vmask_d = nc.declare_dram_parameter("vmask9", [2, N_WIN, 1], F32, isOutput=False)
    rmask_d = nc.declare_dram_parameter("rmask", [2, N_OUT, 1], F32, isOutput=False)
    ones_d = nc.declare_dram_parameter("ones", [N_OUT, 1], F32, isOutput=False)
    part_d = nc.declare_dram_parameter("partial", [1, 1], F32, isOutput=True)

    with tile.TileContext(nc) as tc:
        with (
            tc.tile_pool(name="sb", bufs=1) as sb,
            tc.tile_pool(name="spc", bufs=3) as spc,
            tc.tile_pool(name="sc2", bufs=3) as sc2,
            tc.tile_pool(name="sc1", bufs=1) as sc1,
            tc.tile_pool(name="cst", bufs=1) as cst,
            tc.tile_pool(name="psf", bufs=4, space="PSUM") as psf,
            tc.tile_pool(name="psb", bufs=4, space="PSUM") as psb,
        ):
            band = cst.tile([N_M, N_PIX], F32, name="band", tag="band")
            band_bf = cst.tile([N_M, N_PIX], BF16, name="band_bf", tag="band_bf")
            ones = cst.tile([N_OUT, 1], F32, name="ones", tag="ones")
            nc.sync.dma_start(band[:], band_d[:])
            nc.sync.dma_start(ones[:], ones_d[:])
            nc.vector.tensor_copy(band_bf[:], band[:])

            def fwd_box(dst_ps, src, wm=None, nk=N_PIX, nm=N_WIN):
                # dst[m, j] = sum_{dj} sum_k band[k, m] * src[k, j+dj]
                wm = band if wm is None else wm
                for dj in range(3):
                    nc.tensor.matmul(
                        dst_ps[0:nm, :], wm[0:nk, 0:nm], src[0:nk, dj:dj + NWC],
                        start=(dj == 0), stop=(dj == 2),
                    )

            def t_box(dst_ps, src, c0, c1, wm=None):
                # transpose box: out pixel col j <- window cols j-2, j-1, j
                # out rows [0, 65) <-> global pixel rows [r0, r0+65)
                wm = band if wm is None else wm
                first = True
                for djw in range(3):
                    jlo, jhi = max(c0, djw), min(c1, NWC + djw)
                    nc.tensor.matmul(
                        dst_ps[0:N_OUT, jlo - c0:jhi - c0],
                        wm[0:N_WIN, 0:N_OUT],
                        src[0:N_WIN, jlo - djw:jhi - djw],
                        start=first, stop=(djw == 2),
                    )
                    first = False

            def dil_box(dst_ps, src, c0, c1, wm=None):
                # 3x3 SAME-pad dilate numerator on the pixel grid
                wm = band if wm is None else wm
                first = True
                for dj in (-1, 0, 1):
                    jlo, jhi = max(c0, -dj), min(c1, W - dj)
                    nc.tensor.matmul(
                        dst_ps[0:N_PIX, jlo - c0:jhi - c0],
                        wm[0:N_M, 0:N_PIX],
                        src[0:N_M, jlo + dj:jhi + dj],
                        start=first, stop=(dj == 1),
                    )
                    first = False

            accm_tot = cst.tile([N_OUT, 1], F32, name="accm_tot", tag="accm_tot")

            for s in range(2):
                # ---- load per-sub inputs ----
                img_raw = [sb.tile([N_PIX, W], F32, name=f"imgr{ch}", tag=f"imgr{ch}") for ch in range(3)]
                for ch in range(3):
                    nc.sync.dma_start(img_raw[ch][:], img_d[s, ch])
                ti_a = sb.tile([N_M, W], I32, name="ti_a", tag="ti_a")
                nc.sync.dma_start(ti_a[:], trim_d[s])
                vmask9 = sb.tile([N_WIN, 1], F32, name="vmask9", tag="vmask9")
                rmask = sb.tile([N_OUT, 1], F32, name="rmask", tag="rmask")
                nc.sync.dma_start(vmask9[:], vmask_d[s])
                nc.sync.dma_start(rmask[:], rmask_d[s])

                # ---- per-batch precompute ----
                imgn = [sb.tile([N_PIX, W], F32, name=f"imgn{ch}", tag=f"imgn{ch}") for ch in range(3)]
                for ch in range(3):
                    nc.vector.tensor_scalar(imgn[ch][:], img_raw[ch][:], 1.0 / 255.0, None, OP.mult)
                tf_a = sb.tile([N_M, W], F32, name="tf_a", tag="tf_a")
                nc.vector.tensor_copy(tf_a[:], ti_a[:])
                # out-aligned copies (local row l <-> global pixel row r0+l)
                img_out_raw = [sb.tile([N_OUT, W], F32, name=f"imgor{ch}", tag=f"imgor{ch}")
                               for ch in range(3)]
                imgn_out = [sb.tile([N_OUT, W], F32, name=f"imgno{ch}", tag=f"imgno{ch}")
                            for ch in range(3)]
                for ch in range(3):
                    nc.sync.dma_start(img_out_raw[ch][:], img_d[s, ch, 2:2 + N_OUT])
                    nc.vector.tensor_scalar(imgn_out[ch][:], img_out_raw[ch][:],
                                            1.0 / 255.0, None, OP.mult)
                ti_o = sb.tile([N_OUT, W], I32, name="ti_o", tag="ti_o")
                nc.sync.dma_start(ti_o[:], trim_d[s, 3:3 + N_OUT])
                tf_o = sb.tile([N_OUT, W], F32, name="tf_o", tag="tf_o")
                nc.vector.tensor_copy(tf_o[:], ti_o[:])
                m100o = sb.tile([N_OUT, W], F32, name="m100o", tag="m100o")
                nc.vector.tensor_scalar(m100o[:], tf_o[:], 128.0, TRI_CONF, OP.is_equal, OP.mult)
                mdil = sb.tile([N_M, W], BF16, name="mdil", tag="mdil")
                nc.vector.tensor_scalar(mdil[:], tf_a[:], 128.0, None, OP.is_equal)

                # bf16 copies for the class pipeline
                imgn_bf = [sb.tile([N_PIX, W], BF16, name=f"imgb{ch}", tag=f"imgb{ch}")
                           for ch in range(3)]
                for ch in range(3):
                    nc.scalar.activation(imgn_bf[ch][:], imgn[ch][:],
                                         mybir.ActivationFunctionType.Copy, bias=0.0, scale=1.0)

                # color moments -> mu, E2(+eps on diag); var path stays fp32/exact
                pairs = [(0, 0), (0, 1), (0, 2), (1, 1), (1, 2), (2, 2)]
                mu = [sb.tile([N_WIN, NWC], F32, name=f"mu{ch}", tag=f"mu{ch}") for ch in range(3)]
                mu_bf = [sb.tile([N_WIN, NWC], BF16, name=f"mub{ch}", tag=f"mub{ch}") for ch in range(3)]
                e2 = [sc1.tile([N_WIN, NWC], F32, name=f"e2{i}", tag=f"e2{i}") for i in range(6)]
                for ch in range(3):
                    bps = psf.tile([N_WIN, NWC], F32, name="fwd", tag="fwd")
                    fwd_box(bps, imgn[ch])
                    nc.scalar.activation(mu[ch][:], bps[:], mybir.ActivationFunctionType.Copy,
                                         bias=0.0, scale=1.0 / 9.0)
                    nc.scalar.activation(mu_bf[ch][:], bps[:], mybir.ActivationFunctionType.Copy,
                                         bias=0.0, scale=1.0 / 9.0)
                for i, (a, b) in enumerate(pairs):
                    prod = sc1.tile([N_PIX, W], F32, name="prod", tag="prod")
                    nc.gpsimd.tensor_tensor(prod[:], imgn[a][:], imgn[b][:], OP.mult)
                    bps = psf.tile([N_WIN, NWC], F32, name="fwd", tag="fwd")
                    fwd_box(bps, prod)
                    if a == b:
                        nc.vector.tensor_scalar(e2[i][:], bps[:], 1.0 / 9.0, EPS9, OP.mult, OP.add)
                    else:
                        nc.vector.tensor_scalar(e2[i][:], bps[:], 1.0 / 9.0, None, OP.mult)

                # var = E2 - mu mu^T  (6 unique entries)
                var = [sc1.tile([N_WIN, NWC], F32, name=f"var{i}", tag=f"var{i}") for i in range(6)]
                for i, (a, b) in enumerate(pairs):
                    mm = sc1.tile([N_WIN, NWC], F32, name="mm_sc", tag="mm_sc")
                    nc.gpsimd.tensor_tensor(mm[:], mu[a][:], mu[b][:], OP.mult)
                    nc.gpsimd.tensor_tensor(var[i][:], e2[i][:], mm[:], OP.subtract)
                v11, v12, v13, v22, v23, v33 = var

                # adjugate & inverse
                def fma_sub(x1, y1, x2, y2, tag):
                    # returns x1*y1 - x2*y2
                    p1 = sc1.tile([N_WIN, NWC], F32, name="cof_p1", tag="cof_p1")
                    p2 = sc1.tile([N_WIN, NWC], F32, name="cof_p2", tag="cof_p2")
                    o = sc1.tile([N_WIN, NWC], F32, name=tag, tag=tag)
                    nc.gpsimd.tensor_tensor(p1[:], x1[:], y1[:], OP.mult)
                    nc.gpsimd.tensor_tensor(p2[:], x2[:], y2[:], OP.mult)
                    nc.gpsimd.tensor_tensor(o[:], p1[:], p2[:], OP.subtract)
                    return o

                a11 = fma_sub(v22, v33, v23, v23, "a11")
                a12 = fma_sub(v13, v23, v12, v33, "a12")
                a13 = fma_sub(v12, v23, v13, v22, "a13")
                a22 = fma_sub(v11, v33, v13, v13, "a22")
                a23 = fma_sub(v12, v13, v11, v23, "a23")
                a33 = fma_sub(v11, v22, v12, v12, "a33")
                # det = v11*a11 + v12*a12 + v13*a13
                d1 = sc1.tile([N_WIN, NWC], F32, name="d1", tag="d1")
                d2 = sc1.tile([N_WIN, NWC], F32, name="d2", tag="d2")
                nc.gpsimd.tensor_tensor(d1[:], v11[:], a11[:], OP.mult)
                nc.gpsimd.tensor_tensor(d2[:], v12[:], a12[:], OP.mult)
                nc.gpsimd.tensor_tensor(d1[:], d1[:], d2[:], OP.add)
                nc.gpsimd.tensor_tensor(d2[:], v13[:], a13[:], OP.mult)
                nc.gpsimd.tensor_tensor(d1[:], d1[:], d2[:], OP.add)
                rdet = sc1.tile([N_WIN, NWC], F32, name="rdet", tag="rdet")
                nc.vector.reciprocal(rdet[:], d1[:])
                inv = [sb.tile([N_WIN, NWC], BF16, name=f"inv{i}", tag=f"inv{i}") for i in range(6)]
                for i, adj in enumerate([a11, a12, a13, a22, a23, a33]):
                    nc.vector.tensor_mul(inv[i][:], adj[:], rdet[:])
                i11, i12, i13, i22, i23, i33 = inv

                # keep mask: dilate(~consts) then window-any, then valid/9
                d01 = sb.tile([N_PIX, W], BF16, name="d01", tag="d01")
                for (c0, c1) in ((0, CH0), (CH0, W)):
                    dps = psb.tile([N_PIX, CH0], F32, name="bt", tag="bt")
                    dil_box(dps, mdil, c0, c1, wm=band_bf)
                    nc.vector.tensor_scalar(d01[:, c0:c1], dps[0:N_PIX, 0:c1 - c0], 0.0, None, OP.is_gt)
                keep9 = sb.tile([N_WIN, NWC], BF16, name="keep9", tag="keep9")
                kps = psf.tile([N_WIN, NWC], F32, name="fwd", tag="fwd")
                fwd_box(kps, d01, wm=band_bf)
                nc.vector.tensor_scalar(keep9[:], kps[:], 0.0, vmask9[:], OP.is_gt, OP.mult)

                # Nkc = 9 * box3T(keep9) + 100 - 100*m  (at output pixel rows)
                nkc = sb.tile([N_OUT, W], F32, name="nkc", tag="nkc")
                for (c0, c1) in ((0, CH0), (CH0, W)):
                    nps = psb.tile([N_PIX, CH0], F32, name="bt", tag="bt")
                    t_box(nps, keep9, c0, c1, wm=band_bf)
                    nc.vector.tensor_scalar(nkc[:, c0:c1], nps[0:N_OUT, 0:c1 - c0],
                                            NINE_EFF, TRI_CONF, OP.mult, OP.add)
                nc.vector.tensor_sub(nkc[:], nkc[:], m100o[:])

                acc_w = sb.tile([N_OUT, 16], F32, name="acc_w", tag="acc_w")

                # ---- per-class ----
                for c in range(N_CLASSES):
                    o = spc.tile([N_PIX, W], F32, name="o", tag="o")
                    nc.sync.dma_start(o[:], cpr_d[s, c])
                    o_out = spc.tile([N_OUT, W], F32, name="o_out", tag="o_out")
                    nc.sync.dma_start(o_out[:], cpr_d[s, c, 2:2 + N_OUT])
                    o_bf = spc.tile([N_PIX, W], BF16, name="o_bf", tag="o_bf")
                    nc.scalar.activation(o_bf[:], o[:],
                                         mybir.ActivationFunctionType.Copy, bias=0.0, scale=1.0)

                    sps = psf.tile([N_WIN, NWC], F32, name="fwd", tag="fwd")
                    fwd_box(sps, o_bf, wm=band_bf)
                    qps = []
                    for ch in range(3):
                        po = sc2.tile([N_PIX, W], BF16, name="po", tag="po")
                        nc.vector.tensor_mul(po[:], imgn_bf[ch][:], o_bf[:])
                        qp = psf.tile([N_WIN, NWC], F32, name="fwd", tag="fwd")
                        fwd_box(qp, po, wm=band_bf)
                        qps.append(qp)

                    # bf16 S/q in SBUF (ScalarE copies off PSUM)
                    s_bf = sc2.tile([N_WIN, NWC], BF16, name="s_bf", tag="s_bf")
                    nc.scalar.activation(s_bf[:], sps[:],
                                         mybir.ActivationFunctionType.Copy, bias=0.0, scale=1.0)
                    q_bf = []
                    for ch in range(3):
                        qb = sc2.tile([N_WIN, NWC], BF16, name=f"qb{ch}", tag=f"qb{ch}")
                        nc.scalar.activation(qb[:], qps[ch][:],
                                             mybir.ActivationFunctionType.Copy, bias=0.0, scale=1.0)
                        q_bf.append(qb)

                    # t_ch = q_ch - mu_ch * S
                    t = []
                    for ch in range(3):
                        ms = sc2.tile([N_WIN, NWC], BF16, name="ms", tag="ms")
                        nc.vector.tensor_mul(ms[:], mu_bf[ch][:], s_bf[:])
                        tt = sc2.tile([N_WIN, NWC], BF16, name=f"t{ch}", tag=f"t{ch}")
                        nc.vector.tensor_sub(tt[:], q_bf[ch][:], ms[:])
                        t.append(tt)

                    # v = inv @ t (symmetric)
                    v = []
                    for (ia, ib, ic) in ((i11, i12, i13), (i12, i22, i23), (i13, i23, i33)):
                        vv = sc2.tile([N_WIN, NWC], BF16, name="v_comp", tag="v_comp")
                        p2 = sc2.tile([N_WIN, NWC], BF16, name="v_p2", tag="v_p2")
                        nc.vector.tensor_mul(vv[:], ia[:], t[0][:])
                        nc.vector.tensor_mul(p2[:], ib[:], t[1][:])
                        nc.vector.tensor_add(vv[:], vv[:], p2[:])
                        nc.vector.tensor_mul(p2[:], ic[:], t[2][:])
                        nc.vector.tensor_add(vv[:], vv[:], p2[:])
                        v.append(vv)

                    # muv = mu . v ; ak = (S - muv) * keep9 ; bk_ch = v_ch * keep9
                    muv = sc2.tile([N_WIN, NWC], BF16, name="muv", tag="muv")
                    p2 = sc2.tile([N_WIN, NWC], BF16, name="muv_p2", tag="muv_p2")
                    nc.gpsimd.tensor_tensor(muv[:], mu_bf[0][:], v[0][:], OP.mult)
                    nc.gpsimd.tensor_tensor(p2[:], mu_bf[1][:], v[1][:], OP.mult)
                    nc.gpsimd.tensor_tensor(muv[:], muv[:], p2[:], OP.add)
                    nc.gpsimd.tensor_tensor(p2[:], mu_bf[2][:], v[2][:], OP.mult)
                    nc.gpsimd.tensor_tensor(muv[:], muv[:], p2[:], OP.add)
                    ak = sc2.tile([N_WIN, NWC], BF16, name="ak", tag="ak")
                    nc.vector.tensor_sub(ak[:], s_bf[:], muv[:])
                    nc.vector.tensor_mul(ak[:], ak[:], keep9[:])
                    bk = []
                    for ch in range(3):
                        bb = sc2.tile([N_WIN, NWC], BF16, name=f"bk{ch}", tag=f"bk{ch}")
                        nc.vector.tensor_mul(bb[:], v[ch][:], keep9[:])
                        bk.append(bb)

                    # Res = o*Nkc - A - sum imgn*B - 100*(trimap==c+1); square+reduce
                    for ci, (c0, c1) in enumerate(((0, CH0), (CH0, W))):
                        cw = c1 - c0
                        aps = psb.tile([N_PIX, CH0], F32, name="bt", tag="bt")
                        t_box(aps, ak, c0, c1, wm=band_bf)
                        bps3 = []
                        for ch in range(3):
                            bp = psb.tile([N_PIX, CH0], F32, name="bt", tag="bt")
                            t_box(bp, bk[ch], c0, c1, wm=band_bf)
                            bps3.append(bp)
                        r = sc2.tile([N_OUT, CH0], F32, name="res", tag="res")
                        p = sc2.tile([N_OUT, CH0], F32, name="res_p", tag="res_p")
                        nc.vector.tensor_mul(r[:, 0:cw], o_out[:, c0:c1], nkc[:, c0:c1])
                        nc.vector.tensor_sub(r[:, 0:cw], r[:, 0:cw], aps[0:N_OUT, 0:cw])
                        for ch in range(3):
                            nc.vector.tensor_mul(p[:, 0:cw], imgn_out[ch][:, c0:c1],
                                                 bps3[ch][0:N_OUT, 0:cw])
                            nc.vector.tensor_sub(r[:, 0:cw], r[:, 0:cw], p[:, 0:cw])
                        nc.vector.tensor_scalar(p[:, 0:cw], tf_o[:, c0:c1],
                                                float(c + 1), -TRI_CONF, OP.is_equal, OP.mult)
                        nc.vector.tensor_add(r[:, 0:cw], r[:, 0:cw], p[:, 0:cw])
                        sq = sc2.tile([N_OUT, CH0], F32, name="sq", tag="sq")
                        nc.scalar.activation(sq[:, 0:cw], r[:, 0:cw],
                                             mybir.ActivationFunctionType.Square,
                                             accum_out=acc_w[:, 2 * c + ci:2 * c + ci + 1])

                # ---- reduce this sub ----
                accv = sb.tile([N_OUT, 1], F32, name="accv", tag="accv")
                nc.vector.tensor_reduce(accv[:], acc_w[:, 0:14],
                                        axis=mybir.AxisListType.X, op=OP.add)
                if s == 0:
                    nc.vector.tensor_scalar(accm_tot[:], accv[:], rmask[:], None, OP.mult)
                else:
                    accm = sb.tile([N_OUT, 1], F32, name="accm", tag="accm")
                    nc.vector.tensor_scalar(accm[:], accv[:], rmask[:], None, OP.mult)
                    nc.vector.tensor_add(accm_tot[:], accm_tot[:], accm[:])

            fin_ps = psb.tile([1, 1], F32, name="fin", tag="bt")
            nc.tensor.matmul(fin_ps[:], accm_tot[:], ones[:], start=True, stop=True)
            fin = cst.tile([1, 1], F32, name="fin_sb", tag="fin_sb")
            nc.vector.tensor_copy(fin[:], fin_ps[:])
            nc.sync.dma_start(part_d[:], fin[:])

    nc.compile()
    return nc


def _get_program():
    global _PROGRAM
    if _PROGRAM is None:
        _PROGRAM = _build_program()
    return _PROGRAM


def _host_inputs(cprob, img_org, trimap):
    """Slice + pad full inputs into per-core input maps."""
    cprob = np.ascontiguousarray(cprob, dtype=np.float32)
    img_org = np.ascontiguousarray(img_org, dtype=np.float32)
    trimap = np.ascontiguousarray(trimap, dtype=np.int32)

    band = np.zeros((N_M, N_PIX), np.float32)
    for k in range(N_M):
        for m in range(N_PIX):
            if 0 <= k - m <= 2:
                band[k, m] = 1.0
    ones = np.ones((N_OUT, 1), np.float32)

    def rows(arr, lo, hi, fill):
        # arr[..., lo:hi, :] with zero/fill padding outside [0, H)
        lead = arr.shape[:-2]
        out = np.full(lead + (hi - lo, arr.shape[-1]), fill, arr.dtype)
        alo, ahi = max(lo, 0), min(hi, H)
        if ahi > alo:
            out[..., alo - lo:ahi - lo, :] = arr[..., alo:ahi, :]
        return out

    in_maps = []
    for core in range(8):
        b = core // 4
        subs = (2 * (core % 4), 2 * (core % 4) + 1)
        cpr = np.stack([rows(cprob[b], SUB_R0[s] - 2, SUB_R0[s] + N_PIX - 2, 0.0)
                        for s in subs])
        img = np.stack([rows(np.moveaxis(img_org[b], -1, 0), SUB_R0[s] - 2,
                             SUB_R0[s] + N_PIX - 2, 0.0) for s in subs])
        trm = np.stack([rows(trimap[b], SUB_R0[s] - 3, SUB_R0[s] + N_M - 3, 0)
                        for s in subs])
        vmask = np.zeros((2, N_WIN, 1), np.float32)
        rmask = np.zeros((2, N_OUT, 1), np.float32)
        for i, s in enumerate(subs):
            r0 = SUB_R0[s]
            for l in range(N_WIN):
                if 0 <= r0 - 2 + l < NWC:
                    vmask[i, l, 0] = 1.0 / 9.0
            own = 65 if s == 7 else 64
            rmask[i, 0:own, 0] = 1.0
        in_maps.append({
            "cpr": cpr, "img": img, "trim": trm,
            "band": band, "ones": ones,
            "vmask9": vmask, "rmask": rmask,
        })
    return in_maps


def run(cprob, img_org, trimap, trace=False):
    nc = _get_program()
    in_maps = _host_inputs(cprob, img_org, trimap)
    res = run_bass_kernel_spmd(nc, in_maps, list(range(8)), trace=trace)
    total = sum(float(r["partial"][0, 0]) for r in res.results)
    out = np.float32(total / (float(N) * float(N)))
    return out, res


def kernel(cprob, img_org, trimap):
    out, _ = run(cprob, img_org, trimap)
    return out



# revision 13
# speedup vs baseline: 1.6972x; 1.6972x over previous
"""Trainium2 Bass kernel for nn_ClosedFormLoss (closed-form matting Laplacian loss).

Math (per batch, class): res = o*Nkc - A - sum_ch imgn_ch*B_ch - 100*[tri==c+1],
loss = sum res^2 / n^2, with
    A = box3T(G*S - kw.q),  B_ch = box3T((K@q)_ch - kw_ch*S),  Nk = box3T(keep9)
    S = box3(o), q_ch = box3(imgn_ch*o), K = keep9*inv(var+eps), kw = K@mu,
    G = keep9 + mu.kw     (the keep9/9-weighting and inverse are folded into K).

Geometry: 8 cores = 2 images x 4 row-blocks.  Each core processes ONE fused
pass: a 124-row main block (pixel rows r0-2..r0+126 = 128 partitions exactly)
plus a 17-row x ~129-col "mini" block (image rows 496..513, one column chunk
per core) appended along the free dim of every tile.  Box filters are banded-
matrix matmuls over the partition dim + column-shift accumulation in PSUM.
Moments run as fp32r matmuls (1 cyc/row); the class pipeline is bf16 on DVE
with 3D broadcast APs; the 5x5 keep window is a single banded matmul.

Tile column layout (free dim):
  pixel grid  [128, 656]: main [0:513) pad [513:516) mini [516:649) pad
  window grid [126, 656]: main [0:511) junk 511 gap [512:516) mini [516:647)
  trimap      [128, 656]: main [0:513) pad [513:515) mini [515:650) pad
  out grid    [124, 646]: main [0:513) mini [513:642) pad
"""

import sys
import numpy as np

sys.path.insert(0, "/opt/trn_rl_repo")

from concourse import bacc, mybir, tile  # noqa: E402
from concourse.bass_utils import run_bass_kernel_spmd  # noqa: E402

F32 = mybir.dt.float32
F32R = mybir.dt.float32r
BF16 = mybir.dt.bfloat16
I32 = mybir.dt.int32
OP = mybir.AluOpType
AF = mybir.ActivationFunctionType

N_CLASSES = 7
H = W = 513
NWC = 511
N = H * W
EPS9 = 1e-7 / 9.0
TRI_CONF = 100.0
import ml_dtypes  # noqa: E402
NINE_EFF = float(1.0 / np.float32(np.float32(1.0 / 9.0).astype(ml_dtypes.bfloat16)))

NO = 124            # out rows per main block
NW_ = 126           # window rows
NP_ = 128           # pixel rows (= partition budget exactly)
WT = 656            # pixel/window/trimap tile width
GP = 516            # mini pixel-grid section column offset
GT = 515            # mini trimap section column offset
OUTW = 646          # out tile width
MO = 513            # mini out section offset in out tiles
MCO = [0, 129, 257, 385]
MR0 = 496           # mini block first out row (496..513)

_PROGRAM = None


def _build_program():
    nc = bacc.Bacc("TRN2", target_bir_lowering=False, debug=False, num_devices=8)

    cpr_d = nc.declare_dram_parameter("cpr", [N_CLASSES, NP_, WT], F32, isOutput=False)
    img_d = nc.declare_dram_parameter("img", [3, NP_, WT], F32, isOutput=False)
    trim_d = nc.declare_dram_parameter("trim", [NP_, WT], I32, isOutput=False)
    trimx_d = nc.declare_dram_parameter("trimx", [2, WT], I32, isOutput=False)
    b3_d = nc.declare_dram_parameter("band3", [NP_, NW_], F32, isOutput=False)
    b5_d = nc.declare_dram_parameter("band5", [NP_, NW_], F32, isOutput=False)
    b5x_d = nc.declare_dram_parameter("band5x", [2, NW_], F32, isOutput=False)
    bs2_d = nc.declare_dram_parameter("bsh2", [NP_, NO], F32, isOutput=False)
    bs3_d = nc.declare_dram_parameter("bsh3", [NP_, NO], F32, isOutput=False)
    vm_d = nc.declare_dram_parameter("vmask9", [NW_, 1], F32, isOutput=False)
    km_d = nc.declare_dram_parameter("kmask", [NW_, 137], F32, isOutput=False)
    mm_d = nc.declare_dram_parameter("mmask", [NO, 129], F32, isOutput=False)
    ones_d = nc.declare_dram_parameter("ones", [NO, 1], F32, isOutput=False)
    part_d = nc.declare_dram_parameter("partial", [1, 1], F32, isOutput=True)

    def r3(ap, s):
        # [P, s*WT] slice -> [P, s, 648] 3D AP (drops cols 648: of each section)
        return ap.rearrange("p (s c) -> p s c", s=s)[:, :, 0:648]

    with tile.TileContext(nc) as tc:
        with (
            tc.tile_pool(name="cst", bufs=1) as cst,
            tc.tile_pool(name="sb", bufs=1) as sb,
            tc.tile_pool(name="wk", bufs=14) as wk,
            tc.tile_pool(name="spc", bufs=2) as spc,
            tc.tile_pool(name="spp", bufs=1) as spp,
            tc.tile_pool(name="pf", bufs=2, space="PSUM") as pf,
            tc.tile_pool(name="pt", bufs=4, space="PSUM") as pt,
        ):
            # ---- constants ----
            b3 = cst.tile([NP_, NW_], F32, name="b3", tag="b3")
            b5 = cst.tile([NP_, NW_], F32, name="b5", tag="b5")
            b5x = cst.tile([2, NW_], F32, name="b5x", tag="b5x")
            nc.sync.dma_start(b3[:], b3_d[:])
            nc.sync.dma_start(b5[:], b5_d[:])
            nc.sync.dma_start(b5x[:], b5x_d[:])
            bs2 = cst.tile([NP_, NO], F32, name="bs2", tag="bs2")
            bs3 = cst.tile([NP_, NO], F32, name="bs3", tag="bs3")
            nc.sync.dma_start(bs2[:], bs2_d[:])
            nc.sync.dma_start(bs3[:], bs3_d[:])
            bs2b = cst.tile([NP_, NO], BF16, name="bs2b", tag="bs2b")
            bs3b = cst.tile([NP_, NO], BF16, name="bs3b", tag="bs3b")
            nc.vector.tensor_copy(bs2b[:], bs2[:])
            nc.vector.tensor_copy(bs3b[:], bs3[:])
            b3r = cst.tile([NP_, NW_], F32R, name="b3r", tag="b3r")
            b3b = cst.tile([NP_, NW_], BF16, name="b3b", tag="b3b")
            b5b = cst.tile([NP_, NW_], BF16, name="b5b", tag="b5b")
            b5xb = cst.tile([2, NW_], BF16, name="b5xb", tag="b5xb")
            nc.vector.tensor_copy(b3r[:], b3[:])
            nc.vector.tensor_copy(b3b[:], b3[:])
            nc.vector.tensor_copy(b5b[:], b5[:])
            nc.vector.tensor_copy(b5xb[:], b5x[:])
            vmask9 = cst.tile([NW_, 1], F32, name="vmask9", tag="vmask9")
            nc.sync.dma_start(vmask9[:], vm_d[:])
            kmf = cst.tile([NW_, 137], F32, name="kmf", tag="kmf")
            nc.sync.dma_start(kmf[:], km_d[:])
            kmb = cst.tile([NW_, 137], BF16, name="kmb", tag="kmb")
            nc.vector.tensor_copy(kmb[:], kmf[:])
            mmf = cst.tile([NO, 129], F32, name="mmf", tag="mmf")
            nc.sync.dma_start(mmf[:], mm_d[:])
            mmb = cst.tile([NO, 129], BF16, name="mmb", tag="mmb")
            nc.vector.tensor_copy(mmb[:], mmf[:])
            ones = cst.tile([NO, 1], F32, name="ones", tag="ones")
            nc.sync.dma_start(ones[:], ones_d[:])

            # ---- image inputs ----
            ti = sb.tile([NP_, WT], I32, name="ti", tag="ti")
            tix = sb.tile([2, WT], I32, name="tix", tag="tix")
            nc.sync.dma_start(ti[:], trim_d[:])
            nc.sync.dma_start(tix[:], trimx_d[:])
            tf = sb.tile([NP_, WT], F32, name="tf", tag="tf")
            tfx = sb.tile([2, WT], F32, name="tfx", tag="tfx")
            nc.vector.tensor_copy(tf[:], ti[:])
            nc.vector.tensor_copy(tfx[:], tix[:])
            tf_bf = sb.tile([NP_, WT], BF16, name="tf_bf", tag="tf_bf")
            nc.vector.tensor_copy(tf_bf[:], tf[:])

            def rowshift(dst_bf, src_ap, wmb, n_k=NP_):
                ps = pf.tile([NO, WT], F32, name="shps", tag="pf")
                nc.tensor.matmul(ps[0:NO, 0:512], wmb[0:n_k, :], src_ap[0:n_k, 0:512],
                                 start=True, stop=True)
                nc.tensor.matmul(ps[0:NO, 512:650], wmb[0:n_k, :], src_ap[0:n_k, 512:650],
                                 start=True, stop=True)
                nc.scalar.activation(dst_bf[:, 0:650], ps[0:NO, 0:650],
                                     AF.Copy, bias=0.0, scale=1.0)

            tf_ob = sb.tile([NO, WT], BF16, name="tf_ob", tag="tf_ob")
            rowshift(tf_ob, tf_bf[:], bs3b)

            imgn_r = []
            imgn_bf = sb.tile([NP_, 3 * WT], BF16, name="imgn_bf", tag="imgn_bf")
            for ch in range(3):
                raw = spp.tile([NP_, WT], F32, name="raw", tag="raw", bufs=2)
                nc.sync.dma_start(raw[:], img_d[ch])
                ir = sb.tile([NP_, WT], F32R, name=f"ir{ch}", tag=f"ir{ch}")
                nc.scalar.activation(ir[:], raw[:], AF.Copy, bias=0.0, scale=1.0 / 255.0)
                nc.scalar.activation(imgn_bf[:, ch * WT:(ch + 1) * WT], raw[:],
                                     AF.Copy, bias=0.0, scale=1.0 / 255.0)
                imgn_r.append(ir)
            imgn_ob = sb.tile([NO, 3 * WT], BF16, name="imgn_ob", tag="imgn_ob")
            for ch in range(3):
                rowshift(imgn_ob[:, ch * WT:(ch + 1) * WT],
                         imgn_bf[:, ch * WT:(ch + 1) * WT], bs2b)

            # ---- keep mask: 5x5 OR of (trimap != 128), as one banded matmul ----
            mdil = sb.tile([NP_, WT], BF16, name="mdil", tag="mdil")
            mdilx = sb.tile([2, WT], BF16, name="mdilx", tag="mdilx")
            nc.vector.tensor_scalar(mdil[:], tf_bf[:], 128.0, None, OP.is_equal)
            nc.vector.tensor_scalar(mdilx[:], tfx[:], 128.0, None, OP.is_equal)
            # mdil currently = [tri==128]; keep needs OR of ~consts = [tri==128]
            kp = pf.tile([NW_, WT], F32, name="kp", tag="pf")
            # chunk A: win cols [0:512); dj2=0 is range-deficient -> order it last
            order = [1, 2, 3, 4, 0]
            for k, dj2 in enumerate(order):
                jlo = 1 if dj2 == 0 else 0
                nc.tensor.matmul(kp[0:NW_, jlo:512], b5b[:, :],
                                 mdil[:, jlo - 1 + dj2:511 + dj2],
                                 start=(k == 0), stop=False)
                nc.tensor.matmul(kp[0:NW_, jlo:512], b5xb[:, :],
                                 mdilx[:, jlo - 1 + dj2:511 + dj2],
                                 start=False, stop=(k == len(order) - 1))
            for k, dj2 in enumerate(range(5)):
                nc.tensor.matmul(kp[0:NW_, 512:650], b5b[:, :],
                                 mdil[:, 511 + dj2:649 + dj2],
                                 start=(k == 0), stop=False)
                nc.tensor.matmul(kp[0:NW_, 512:650], b5xb[:, :],
                                 mdilx[:, 511 + dj2:649 + dj2],
                                 start=False, stop=(k == 4))
            keep9 = sb.tile([NW_, WT], BF16, name="keep9", tag="keep9")
            nc.vector.tensor_scalar(keep9[:, 0:512], kp[0:NW_, 0:512],
                                    0.0, vmask9[:], OP.is_gt, OP.mult)
            nc.vector.tensor_scalar(keep9[:, 512:650], kp[0:NW_, 512:650],
                                    0.0, 1.0 / 9.0, OP.is_gt, OP.mult)
            nc.vector.tensor_tensor(keep9[:, 511:648], keep9[:, 511:648],
                                    kmb[:], OP.mult)

            # ---- color moments (fp32r matmuls) ----
            def fwd(dst_ps, src, wm, n_k=NP_):
                for dj in range(3):
                    nc.tensor.matmul(dst_ps[0:NW_, 0:512], wm[0:n_k, :],
                                     src[0:n_k, dj:dj + 512],
                                     start=(dj == 0), stop=(dj == 2))
                for dj in range(3):
                    nc.tensor.matmul(dst_ps[0:NW_, 512:650], wm[0:n_k, :],
                                     src[0:n_k, 512 + dj:650 + dj],
                                     start=(dj == 0), stop=(dj == 2))

            mu = sb.tile([NW_, 3 * WT], F32, name="mu", tag="mu")
            mu_bf = sb.tile([NW_, 3 * WT], BF16, name="mu_bf", tag="mu_bf")
            for ch in range(3):
                ps = pf.tile([NW_, WT], F32, name="mps", tag="pf")
                fwd(ps, imgn_r[ch][:], b3r)
                nc.scalar.activation(mu[:, ch * WT:ch * WT + 648], ps[0:NW_, 0:648],
                                     AF.Copy, bias=0.0, scale=1.0 / 9.0)
                nc.scalar.activation(mu_bf[:, ch * WT:ch * WT + 648], ps[0:NW_, 0:648],
                                     AF.Copy, bias=0.0, scale=1.0 / 9.0)
            pairs = [(0, 0), (0, 1), (0, 2), (1, 1), (1, 2), (2, 2)]
            e2 = []
            for i, (a, b) in enumerate(pairs):
                prod = spp.tile([NP_, WT], F32R, name="prod", tag="prod", bufs=2)
                nc.vector.tensor_tensor(prod[:], imgn_r[a][:], imgn_r[b][:], OP.mult)
                ps = pf.tile([NW_, WT], F32, name="eps_", tag="pf")
                fwd(ps, prod[:], b3r)
                e = wk.tile([NW_, WT], F32, name=f"e2_{i}", tag="wk")
                nc.scalar.activation(e[:, 0:648], ps[0:NW_, 0:648], AF.Copy,
                                     bias=(EPS9 if a == b else 0.0), scale=1.0 / 9.0)
                e2.append(e)

            # ---- var = E2 - mu mu^T ; adjugate; det; K = adj * (rdet*keep9) ----
            var = []
            for i, (a, b) in enumerate(pairs):
                mm = wk.tile([NW_, WT], F32, name="mmv", tag="wk")
                nc.gpsimd.tensor_tensor(mm[:, 0:648], mu[:, a * WT:a * WT + 648],
                                        mu[:, b * WT:b * WT + 648], OP.mult)
                v = wk.tile([NW_, WT], F32, name=f"var{i}", tag="wk")
                nc.gpsimd.tensor_tensor(v[:, 0:648], e2[i][:, 0:648], mm[:, 0:648],
                                        OP.subtract)
                var.append(v)
            v11, v12, v13, v22, v23, v33 = var

            def fma_sub(x1, y1, x2, y2, nm):
                p1 = wk.tile([NW_, WT], F32, name="cp1", tag="wk")
                p2 = wk.tile([NW_, WT], F32, name="cp2", tag="wk")
                o = wk.tile([NW_, WT], F32, name=nm, tag="wk")
                nc.vector.tensor_tensor(p1[:, 0:648], x1[:, 0:648], y1[:, 0:648], OP.mult)
                nc.vector.tensor_tensor(p2[:, 0:648], x2[:, 0:648], y2[:, 0:648], OP.mult)
                nc.vector.tensor_tensor(o[:, 0:648], p1[:, 0:648], p2[:, 0:648], OP.subtract)
                return o

            a11 = fma_sub(v22, v33, v23, v23, "a11")
            a12 = fma_sub(v13, v23, v12, v33, "a12")
            a13 = fma_sub(v12, v23, v13, v22, "a13")
            a22 = fma_sub(v11, v33, v13, v13, "a22")
            a23 = fma_sub(v12, v13, v11, v23, "a23")
            a33 = fma_sub(v11, v22, v12, v12, "a33")
            d1 = wk.tile([NW_, WT], F32, name="d1", tag="wk")
            d2 = wk.tile([NW_, WT], F32, name="d2", tag="wk")
            nc.gpsimd.tensor_tensor(d1[:, 0:648], v11[:, 0:648], a11[:, 0:648], OP.mult)
            nc.gpsimd.tensor_tensor(d2[:, 0:648], v12[:, 0:648], a12[:, 0:648], OP.mult)
            nc.gpsimd.tensor_tensor(d1[:, 0:648], d1[:, 0:648], d2[:, 0:648], OP.add)
            nc.gpsimd.tensor_tensor(d2[:, 0:648], v13[:, 0:648], a13[:, 0:648], OP.mult)
            nc.gpsimd.tensor_tensor(d1[:, 0:648], d1[:, 0:648], d2[:, 0:648], OP.add)
            rdet = sb.tile([NW_, WT], F32, name="rdet", tag="rdet")
            nc.vector.reciprocal(rdet[:, 0:648], d1[:, 0:648])
            rk = sb.tile([NW_, WT], BF16, name="rk", tag="rk")
            nc.vector.tensor_tensor(rk[:, 0:648], rdet[:, 0:648], keep9[:, 0:648], OP.mult)

            K9 = sb.tile([NW_, 9 * WT], BF16, name="K9", tag="K9")
            for s, adj in ((0, a11), (1, a12), (2, a13), (4, a22), (5, a23), (8, a33)):
                nc.vector.tensor_tensor(K9[:, s * WT:s * WT + 648], adj[:, 0:648],
                                        rk[:, 0:648], OP.mult)
            for s, t in ((3, 1), (6, 2), (7, 5)):
                nc.vector.tensor_copy(K9[:, s * WT:s * WT + 648],
                                      K9[:, t * WT:t * WT + 648])

            # kw = K @ mu ; G = keep9 + mu . kw
            kw = sb.tile([NW_, 3 * WT], BF16, name="kw", tag="kw")
            mu3 = r3(mu_bf[0:NW_, 0:3 * WT], 3)
            for i in range(3):
                P = spp.tile([NW_, 3 * WT], BF16, name="Pkw", tag="P")
                nc.vector.tensor_tensor(r3(P[0:NW_, 0:3 * WT], 3),
                                        r3(K9[0:NW_, 3 * i * WT:(3 * i + 3) * WT], 3),
                                        mu3, OP.mult)
                nc.vector.tensor_tensor(kw[:, i * WT:i * WT + 648], P[:, 0:648],
                                        P[:, WT:WT + 648], OP.add)
                nc.vector.tensor_tensor(kw[:, i * WT:i * WT + 648],
                                        kw[:, i * WT:i * WT + 648],
                                        P[:, 2 * WT:2 * WT + 648], OP.add)
            G = sb.tile([NW_, WT], BF16, name="G", tag="G")
            Pg = spp.tile([NW_, 3 * WT], BF16, name="Pg", tag="P")
            nc.vector.tensor_tensor(r3(Pg[0:NW_, 0:3 * WT], 3), mu3,
                                    r3(kw[0:NW_, 0:3 * WT], 3), OP.mult)
            nc.vector.tensor_tensor(G[:, 0:648], Pg[:, 0:648], Pg[:, WT:WT + 648], OP.add)
            nc.vector.tensor_tensor(G[:, 0:648], G[:, 0:648], Pg[:, 2 * WT:2 * WT + 648], OP.add)
            nc.vector.tensor_tensor(G[:, 0:648], G[:, 0:648], keep9[:, 0:648], OP.add)

            # ---- Nkc = NINE_EFF * box3T(keep9) + 100 - 100*[tri==128] ----
            def tbox1(dst_ps, src_ap_fn, o0, w, full_dst_off=0):
                # single-source transpose box: out col j <- win cols j-2..j
                for djw in range(3):
                    jlo = max(o0, djw) if o0 < 513 else o0
                    nc.tensor.matmul(
                        dst_ps[0:NO, full_dst_off + (jlo - o0):full_dst_off + w],
                        b3b[0:NW_, 0:NO], src_ap_fn(jlo, o0 + w, djw),
                        start=(djw == 0), stop=(djw == 2))

            nkc = sb.tile([NO, OUTW], BF16, name="nkc", tag="nkc")
            # main [0:512), [512:513), mini [513:642)
            pnk = pt.tile([NO, 512], F32, name="pnk", tag="pt")
            tbox1(pnk, lambda jl, jh, dw: keep9[0:NW_, jl - dw:jh - dw], 0, 512)
            nc.vector.tensor_scalar(nkc[:, 0:512], pnk[0:NO, 0:512],
                                    NINE_EFF, TRI_CONF, OP.mult, OP.add)
            pn1 = pt.tile([NO, 4], F32, name="pn1", tag="pt")
            for djw in range(3):
                nc.tensor.matmul(pn1[0:NO, 0:1], b3b[0:NW_, 0:NO],
                                 keep9[0:NW_, 512 - djw:513 - djw],
                                 start=(djw == 0), stop=(djw == 2))
            nc.vector.tensor_scalar(nkc[:, 512:513], pn1[0:NO, 0:1],
                                    NINE_EFF, TRI_CONF, OP.mult, OP.add)
            pnm = pt.tile([NO, 512], F32, name="pnm", tag="pt")
            for djw in range(3):
                nc.tensor.matmul(pnm[0:NO, 0:129], b3b[0:NW_, 0:NO],
                                 keep9[0:NW_, 518 - djw:647 - djw],
                                 start=(djw == 0), stop=(djw == 2))
            nc.vector.tensor_scalar(nkc[:, 513:642], pnm[0:NO, 0:129],
                                    NINE_EFF, TRI_CONF, OP.mult, OP.add)
            m100 = sb.tile([NO, OUTW], BF16, name="m100", tag="m100")
            nc.vector.tensor_scalar(m100[:, 0:513], tf_ob[:, 0:513],
                                    128.0, TRI_CONF, OP.is_equal, OP.mult)
            nc.vector.tensor_scalar(m100[:, 513:642], tf_ob[:, 518:647],
                                    128.0, TRI_CONF, OP.is_equal, OP.mult)
            nc.vector.tensor_tensor(nkc[:, 0:642], nkc[:, 0:642], m100[:, 0:642],
                                    OP.subtract)

            acc_w = sb.tile([NO, 8], F32, name="acc_w", tag="acc_w")

            # ---- per-class pipeline ----
            for c in range(N_CLASSES):
                o = spc.tile([NP_, WT], F32, name="o", tag="o")
                nc.sync.dma_start(o[:], cpr_d[c])
                o_bf = spc.tile([NP_, WT], BF16, name="o_bf", tag="o_bf")
                nc.scalar.activation(o_bf[:], o[:], AF.Copy, bias=0.0, scale=1.0)
                o_ob = spc.tile([NO, WT], BF16, name="o_ob", tag="o_ob")
                rowshift(o_ob, o_bf[:], bs2b)
                po = spc.tile([NP_, 3 * WT], BF16, name="po", tag="po")
                nc.vector.tensor_tensor(
                    po[0:NP_, 0:3 * WT].rearrange("p (s c) -> p s c", s=3),
                    imgn_bf[0:NP_, 0:3 * WT].rearrange("p (s c) -> p s c", s=3),
                    o_bf[0:NP_, 0:WT].unsqueeze(1).broadcast_to((NP_, 3, WT)),
                    OP.mult)

                # S, q0..2 = box3 of [o, po0, po1, po2]
                scat = spc.tile([NW_, 4 * WT], BF16, name="scat", tag="scat")
                for k in range(4):
                    src = o_bf[:] if k == 0 else po[:, (k - 1) * WT:k * WT]
                    ps = pf.tile([NW_, WT], F32, name="fps", tag="pf")
                    fwd(ps, src, b3b)
                    nc.scalar.activation(scat[:, k * WT:k * WT + 648],
                                         ps[0:NW_, 0:648], AF.Copy, bias=0.0, scale=1.0)
                S = scat[0:NW_, 0:WT]
                q3 = r3(scat[0:NW_, WT:4 * WT], 3)

                # u = K @ q ; bk = u - kw*S ; muvk = kw . q ; ak = G*S - muvk
                u = spc.tile([NW_, 3 * WT], BF16, name="u", tag="u")
                for i in range(3):
                    P = spp.tile([NW_, 3 * WT], BF16, name="Pu", tag="P")
                    nc.vector.tensor_tensor(r3(P[0:NW_, 0:3 * WT], 3),
                                            r3(K9[0:NW_, 3 * i * WT:(3 * i + 3) * WT], 3),
                                            q3, OP.mult)
                    nc.vector.tensor_tensor(u[:, i * WT:i * WT + 648], P[:, 0:648],
                                            P[:, WT:WT + 648], OP.add)
                    nc.vector.tensor_tensor(u[:, i * WT:i * WT + 648],
                                            u[:, i * WT:i * WT + 648],
                                            P[:, 2 * WT:2 * WT + 648], OP.add)

                vk = spc.tile([NW_, 4 * WT], BF16, name="vk", tag="vk")
                Pk = spp.tile([NW_, 3 * WT], BF16, name="Pks", tag="P")
                nc.vector.tensor_tensor(
                    r3(Pk[0:NW_, 0:3 * WT], 3), r3(kw[0:NW_, 0:3 * WT], 3),
                    S.unsqueeze(1).broadcast_to((NW_, 3, WT))[:, :, 0:648], OP.mult)
                nc.vector.tensor_tensor(r3(vk[0:NW_, WT:4 * WT], 3),
                                        r3(u[0:NW_, 0:3 * WT], 3),
                                        r3(Pk[0:NW_, 0:3 * WT], 3), OP.subtract)
                Pm = spp.tile([NW_, 3 * WT], BF16, name="Pm", tag="P")
                nc.vector.tensor_tensor(r3(Pm[0:NW_, 0:3 * WT], 3),
                                        r3(kw[0:NW_, 0:3 * WT], 3), q3, OP.mult)
                muvk = spc.tile([NW_, WT], BF16, name="muvk", tag="muvk")
                nc.vector.tensor_tensor(muvk[:, 0:648], Pm[:, 0:648],
                                        Pm[:, WT:WT + 648], OP.add)
                nc.vector.tensor_tensor(muvk[:, 0:648], muvk[:, 0:648],
                                        Pm[:, 2 * WT:2 * WT + 648], OP.add)
                nc.vector.tensor_tensor(vk[:, 0:648], G[:, 0:648], S[:, 0:648], OP.mult)
                nc.vector.tensor_tensor(vk[:, 0:648], vk[:, 0:648],
                                        muvk[:, 0:648], OP.subtract)

                # A,B = box3T of [ak, bk0..2]  (batched 4-source matmuls)
                AB = spc.tile([NO, 4 * OUTW], BF16, name="AB", tag="AB")
                vk4 = vk[0:NW_, 0:4 * WT].rearrange("p (s c) -> p s c", s=4)
                ab4 = AB[0:NO, 0:4 * OUTW].rearrange("p (s c) -> p s c", s=4)

                def tbox4(w, movlo_fn, aboff):
                    psb = pt.tile([NO, 512], F32, name="ptb", tag="pt")
                    p4 = psb[0:NO, 0:4 * w].rearrange("p (s c) -> p s c", s=4)
                    for djw in range(3):
                        lo, ww = movlo_fn(djw)
                        nc.tensor.matmul(p4[:, :, (w - ww):w], b3b[0:NW_, 0:NO],
                                         vk4[:, :, lo:lo + ww],
                                         start=(djw == 0), stop=(djw == 2))
                    nc.scalar.activation(ab4[:, :, aboff:aboff + w], p4[:, :, 0:w],
                                         AF.Copy, bias=0.0, scale=1.0)

                for ci in range(4):
                    o0 = 128 * ci
                    tbox4(128, lambda dw, o0=o0: (max(o0, dw) - dw,
                                                  128 - (max(o0, dw) - o0)), o0)
                tbox4(1, lambda dw: (512 - dw, 1), 512)
                tbox4(128, lambda dw: (518 - dw, 128), 513)
                tbox4(1, lambda dw: (646 - dw, 1), 641)

                # res = o*nkc - A - sum_ch imgn*B - 100*[tri==c+1]
                res = spc.tile([NO, OUTW], BF16, name="res", tag="res")
                ib = spc.tile([NO, 3 * OUTW], BF16, name="ib", tag="ib")
                for (olo, ohi, plo, phi) in ((0, 513, 0, 513), (513, 642, 518, 647)):
                    w = ohi - olo
                    nc.vector.tensor_tensor(res[:, olo:ohi], o_ob[:, plo:phi],
                                            nkc[:, olo:ohi], OP.mult)
                    nc.vector.tensor_tensor(res[:, olo:ohi], res[:, olo:ohi],
                                            AB[:, olo:ohi], OP.subtract)
                    nc.vector.tensor_tensor(
                        ib[0:NO, 0:3 * OUTW].rearrange("p (s c) -> p s c", s=3)[:, :, olo:ohi],
                        imgn_ob[0:NO, 0:3 * WT].rearrange("p (s c) -> p s c", s=3)[:, :, plo:phi],
                        AB[0:NO, 0:4 * OUTW].rearrange("p (s c) -> p s c", s=4)[:, 1:4, olo:ohi],
                        OP.mult)
                    for ch in range(3):
                        nc.vector.tensor_tensor(res[:, olo:ohi], res[:, olo:ohi],
                                                ib[:, ch * OUTW + olo:ch * OUTW + ohi],
                                                OP.subtract)
                    tt = spc.tile([NO, OUTW], BF16, name="tt", tag="tt")
                    nc.vector.tensor_scalar(tt[:, olo:ohi], tf_ob[:, plo:phi],
                                            float(c + 1), -TRI_CONF, OP.is_equal, OP.mult)
                    nc.vector.tensor_tensor(res[:, olo:ohi], res[:, olo:ohi],
                                            tt[:, olo:ohi], OP.add)
                nc.vector.tensor_tensor(res[:, 513:642], res[:, 513:642], mmb[:], OP.mult)
                sq = spc.tile([NO, OUTW], BF16, name="sq", tag="sq")
                nc.scalar.activation(sq[:, 0:642], res[:, 0:642], AF.Square,
                                     accum_out=acc_w[:, c:c + 1])

            accv = sb.tile([NO, 1], F32, name="accv", tag="accv")
            nc.vector.tensor_reduce(accv[:], acc_w[:, 0:N_CLASSES],
                                    axis=mybir.AxisListType.X, op=OP.add)
            fin_ps = pt.tile([1, 4], F32, name="fin", tag="pt")
            nc.tensor.matmul(fin_ps[0:1, 0:1], accv[:], ones[:], start=True, stop=True)
            fin = cst.tile([1, 1], F32, name="fin_sb", tag="fin_sb")
            nc.vector.tensor_copy(fin[:], fin_ps[0:1, 0:1])
            nc.sync.dma_start(part_d[:], fin[:])

    nc.compile()
    return nc


def _get_program():
    global _PROGRAM
    if _PROGRAM is None:
        _PROGRAM = _build_program()
    return _PROGRAM


def _pad_rows_cols(src, rlo, rhi, clo, chi, dtype):
    """src[rlo:rhi, clo:chi] with zero padding outside bounds; leading dims kept."""
    lead = src.shape[:-2]
    out = np.zeros(lead + (rhi - rlo, chi - clo), dtype)
    arlo, arhi = max(rlo, 0), min(rhi, src.shape[-2])
    aclo, achi = max(clo, 0), min(chi, src.shape[-1])
    if arhi > arlo and achi > aclo:
        out[..., arlo - rlo:arhi - rlo, aclo - clo:achi - clo] = \
            src[..., arlo:arhi, aclo:achi]
    return out


def _host_inputs(cprob, img_org, trimap):
    cprob = np.ascontiguousarray(cprob, dtype=np.float32)
    img_ch = np.ascontiguousarray(np.moveaxis(img_org, -1, 1), dtype=np.float32)  # [2,3,H,W]
    trimap = np.ascontiguousarray(trimap, dtype=np.int32)

    b3 = np.zeros((NP_, NW_), np.float32)
    b5 = np.zeros((NP_, NW_), np.float32)
    b5x = np.zeros((2, NW_), np.float32)
    for k in range(NP_):
        for m in range(NW_):
            if 0 <= k - m <= 2:
                b3[k, m] = 1.0
            if 0 <= k - m <= 4:
                b5[k, m] = 1.0
    for j in range(2):
        for m in range(NW_):
            if 0 <= 128 + j - m <= 4:
                b5x[j, m] = 1.0
    bsh2 = np.zeros((NP_, NO), np.float32)
    bsh3 = np.zeros((NP_, NO), np.float32)
    for m in range(NO):
        bsh2[m + 2, m] = 1.0
        bsh3[m + 3, m] = 1.0
    ones = np.ones((NO, 1), np.float32)

    in_maps = []
    for core in range(8):
        b, i = core // 4, core % 4
        r0 = 124 * i
        co = MCO[i]
        ow = 129 if i == 0 else 128

        def pack_pix(src):  # [..., H, W] -> [..., 128, 656]
            lead = src.shape[:-2]
            out = np.zeros(lead + (NP_, WT), np.float32)
            out[..., :, 0:513] = _pad_rows_cols(src, r0 - 2, r0 + 126, 0, 513, np.float32)
            out[..., 0:21, GP:GP + 133] = _pad_rows_cols(src, MR0 - 2, MR0 + 19,
                                                         co - 2, co + 131, np.float32)
            return out

        cpr = pack_pix(cprob[b])
        img = pack_pix(img_ch[b])
        trim = np.zeros((NP_, WT), np.int32)
        trim[:, 0:513] = _pad_rows_cols(trimap[b], r0 - 3, r0 + 125, 0, 513, np.int32)
        trim[0:23, GT:GT + 135] = _pad_rows_cols(trimap[b], MR0 - 3, MR0 + 20,
                                                 co - 3, co + 132, np.int32)
        trimx = np.zeros((2, WT), np.int32)
        trimx[:, 0:513] = _pad_rows_cols(trimap[b], r0 + 125, r0 + 127, 0, 513, np.int32)

        vmask9 = np.zeros((NW_, 1), np.float32)
        for l in range(NW_):
            if 0 <= r0 - 2 + l < NWC:
                vmask9[l, 0] = 1.0 / 9.0
        kmask = np.zeros((NW_, 137), np.float32)
        for l in range(17):
            for x in range(131):
                if 0 <= co - 2 + x < NWC:
                    kmask[l, 5 + x] = 1.0
        mmask = np.zeros((NO, 129), np.float32)
        mmask[0:17, 0:ow] = 1.0

        in_maps.append({
            "cpr": cpr, "img": img, "trim": trim, "trimx": trimx,
            "band3": b3, "band5": b5, "band5x": b5x,
            "bsh2": bsh2, "bsh3": bsh3,
            "vmask9": vmask9, "kmask": kmask, "mmask": mmask, "ones": ones,
        })
    return in_maps


def run(cprob, img_org, trimap, trace=False):
    nc = _get_program()
    in_maps = _host_inputs(cprob, img_org, trimap)
    res = run_bass_kernel_spmd(nc, in_maps, list(range(8)), trace=trace)
    total = sum(float(r["partial"][0, 0]) for r in res.results)
    out = np.float32(total / (float(N) * float(N)))
    return out, res


def kernel(cprob, img_org, trimap):
    out, _ = run(cprob, img_org, trimap)
    return out
